# revision 3
# baseline (speedup 1.0000x reference)
"""Trainium2 Bass kernel for a 12-head attention block with post-softmax
additive per-head bias.

    qkv = x @ W_qkv                          x: [64, 196, 768]
    attn = softmax(q k^T / 8) + static_a     (bias added AFTER softmax)
    out = (attn @ v) @ W_proj + b_proj

Sharding: data-parallel over batch across 8 NeuronCores (8 batches each).
No collectives. Weights replicated.

Numerics: static_a (~0.9/entry) dominates the softmax probabilities
(~0.005..0.05/entry) in attn@v — the softmax path carries only ~1.2% of
the output norm, so q/k/S can run in fp8-e4m3 while v / A@v / proj stay
bf16. fp8 DoubleRow matmuls stream 2 K-tiles per instruction at 0.5
cyc/row (4x bf16 on the contraction-heavy qkT).

Per-core dataflow:
  prologue: q^T,k^T = W_qk^T @ x^T for ALL 8 local batches at once
            (fp8 DoubleRow, K=768 as 3 pairs of 128) -> fp8 SBUF
  per batch b:
    v(b)   = x_b @ W_v                 (bf16, 65-stride + ones column)
    S^T(b) = k q^T (fp8) -> exp (ACT, scale=1/(8*256), bias=-ln16) -> P~^T
    AV(b)  = A_h @ v  (bf16, per-head static bias term)
    U(b)   = P~ @ [v|1]  (ones column gives softmax row sums)
    O(b)   = U * (1/r) + AV            (DVE)
    O^T(b) via PE transpose -> attn_outT
  out = attn_out @ W_proj  (forward; b_proj added on host — it is zeros)
"""

import math
import os
import sys

_TRN_REPO = "/opt/trn_rl_repo"
if _TRN_REPO not in sys.path:
    sys.path.insert(0, _TRN_REPO)

import numpy as np
import ml_dtypes

import concourse.bass as bass
import concourse.tile as tile
from concourse import bacc, mybir
from concourse.bass import MemorySpace
from concourse.bass_utils import run_bass_kernel_spmd
from concourse.masks import make_identity

BF16 = mybir.dt.bfloat16
F32 = mybir.dt.float32
F8 = mybir.dt.float8e4
DR = mybir.MatmulPerfMode.DoubleRow

N_CORES = 8
BATCH = 64
B = BATCH // N_CORES  # 8 local batches per core
H = 12
D = 64
N = 196
C = 768
T = B * N  # 1568 local tokens
KC = 6  # contraction chunks of 128 over C=768
W8SCALE = 16.0  # host pre-scale on W_q,W_k before fp8 quantization
SCALE = (D ** -0.5) / (W8SCALE * W8SCALE)  # folded into exp
PSHIFT = 16.0  # P~ stored as exp(.)/PSHIFT to stay in fp8 range
EXPBIAS = -math.log(PSHIFT)

# token chunks of 128 over T (for the projection)
MCS = [(i * 128, min(128, T - i * 128)) for i in range((T + 127) // 128)]
# per-batch row chunks over N=196
NCH = [(0, 128), (128, 68)]
# qkT free-dim chunks over T (PSUM bank = 512 f32)
QKC = [(i * 512, min(512, T - i * 512)) for i in range((T + 511) // 512)]

AluOp = mybir.AluOpType
ActFn = mybir.ActivationFunctionType


def _emit(nc: bass.Bass):
    # xt8: fp8 x^T in DR layout: xt8[p, kp, kt, t] = x[t, (2kp+kt)*128+p]
    # xtb: bf16 x^T: xtb[p, kc, t] = x[t, kc*128+p]
    # w8:  fp8 16*W_qk: w8[p, kp, kt, f] = 16*W_qkv[(2kp+kt)*128+p, f]
    # wv:  bf16 W_v: wv[p, kc, f] = W_qkv[kc*128+p, 2C+f]
    # at:  bf16 A^T: at[mc, p, h*N+n] = A[h, n, mc*128+p]
    # wp:  bf16 W_proj: wp[p, kc, o] = W_proj[kc*128+p, o]
    xt8_d = nc.declare_dram_parameter("xt8", [128, 6 * T], F8, isOutput=False)
    xtb_d = nc.declare_dram_parameter("xtb", [128, 6 * T], BF16, isOutput=False)
    w8_d = nc.declare_dram_parameter("w8", [128, 6 * 2 * C], F8, isOutput=False)
    wv_d = nc.declare_dram_parameter("wv", [128, KC * C], BF16, isOutput=False)
    at_d = nc.declare_dram_parameter("at", [2, 128, H * N], BF16, isOutput=False)
    wp_d = nc.declare_dram_parameter("wp", [128, KC * C], BF16, isOutput=False)
    out_d = nc.declare_dram_parameter("out", [T, C], F32, isOutput=True)

    with tile.TileContext(nc) as tc:
        from contextlib import ExitStack

        with ExitStack() as stk:
            const = stk.enter_context(tc.tile_pool(name="const", bufs=1))
            wq = stk.enter_context(tc.tile_pool(name="wq", bufs=1))
            qkp = stk.enter_context(tc.tile_pool(name="qkp", bufs=1))
            vbp = stk.enter_context(tc.tile_pool(name="vbp", bufs=4))
            obp = stk.enter_context(tc.tile_pool(name="obp", bufs=4))
            ptp = stk.enter_context(tc.tile_pool(name="ptp", bufs=6))
            small = stk.enter_context(tc.tile_pool(name="small", bufs=3))
            outst = stk.enter_context(tc.tile_pool(name="outst", bufs=4))
            aotp = stk.enter_context(tc.tile_pool(name="aotp", bufs=1))

            # ---- constants ----
            ident = const.tile([128, 128], BF16)
            make_identity(nc, ident)
            ebias = const.tile([128, 1], F32)
            nc.vector.memset(ebias, EXPBIAS)

            w8_sb = wq.tile([128, 3, 2, 2 * C], F8)
            wv_sb = wq.tile([128, KC, C], BF16)
            xt8_sb = wq.tile([128, 3, 2, T], F8)
            xtb_sb = wq.tile([128, KC, T], BF16)
            at_sb = const.tile([128, 2, H * N], BF16)
            wp_sb = const.tile([128, KC, C], BF16)
            aot_sb = aotp.tile([128, KC, T], BF16)

            # q^T/k^T fp8, feature-major, all batches: q8[p, ft, t]
            q8 = qkp.tile([128, KC, T], F8)
            k8 = qkp.tile([128, KC, T], F8)
            # odd heads' d-rows 64..127 staged to base partition 0
            sq8 = qkp.tile([64, KC, T], F8)
            sk8 = qkp.tile([64, KC, T], F8)

            vb_t = {}
            ob_t = {}
            pt_t = {}

            def emit_qkT(psA, psB):
                # q^T,k^T = W_qk^T @ x^T for all 1568 tokens, fp8 DoubleRow.
                # dst tiles are [128, KC, T]; ft 0..5 -> q, 6..11 -> k.
                for ft in range(12):
                    dst = q8 if ft < 6 else k8
                    c = ft % 6
                    for qc, (toff, tlen) in enumerate(QKC):
                        if (ft + qc) % 2 == 0:
                            pq = psA.tile([128, 512], F32, tag="pA")
                        else:
                            pq = psB.tile([128, 1024], F32, tag="pB")
                        for kp in range(3):
                            nc.tensor.matmul(
                                pq[:, 0:tlen],
                                lhsT=w8_sb[:, kp, :, ft * 128 : ft * 128 + 128],
                                rhs=xt8_sb[:, kp, :, toff : toff + tlen],
                                start=(kp == 0),
                                stop=(kp == 2),
                                perf_mode=DR,
                            )
                        if ft % 2 == 0:
                            nc.vector.tensor_copy(
                                dst[:, c, toff : toff + tlen], pq[:, 0:tlen]
                            )
                        else:
                            nc.scalar.copy(
                                dst[:, c, toff : toff + tlen], pq[:, 0:tlen]
                            )
                    # stage odd-head rows down to partition 0 as soon as a
                    # feature-tile is complete
                    stg = sq8 if ft < 6 else sk8
                    nc.sync.dma_start(
                        out=stg[:, c, :], in_=dst[64:128, c, :]
                    )

            def emit_v(b, psA):
                vb = vbp.tile([128, 2, H * 65], BF16, tag="vb")
                vb_t[b] = vb
                for mc, (moff, mlen) in enumerate(NCH):
                    vv = vb[0:mlen, mc, :].rearrange("p (h x) -> p h x", h=H)
                    nc.vector.memset(vv[:, :, 64:65], 1.0)
                    for ns in range(2):
                        ps = psA.tile([128, 512], F32, tag="pA")
                        for kc in range(KC):
                            nc.tensor.matmul(
                                ps[0:mlen, 0:384],
                                lhsT=xtb_sb[
                                    :, kc, b * N + moff : b * N + moff + mlen
                                ],
                                rhs=wv_sb[:, kc, ns * 384 : (ns + 1) * 384],
                                start=(kc == 0),
                                stop=(kc == KC - 1),
                            )
                        nc.vector.tensor_copy(
                            vv[:, ns * 6 : (ns + 1) * 6, 0:64],
                            ps[0:mlen, 0:384].rearrange("p (h c) -> p h c", h=6),
                        )

            def emit_st_av(b, psB):
                # S^T then exp on ACT; A@v blocks interleaved to keep PE fed
                vb = vb_t[b]
                ob = obp.tile([128, 2, C], BF16, tag="ob")
                ob_t[b] = ob

                def st_unit(mc, hg):
                    moff, mlen = NCH[mc]
                    pt = pt_t[(b, mc)]
                    ps = psB.tile([128, 1024], F32, tag="pB")
                    for hh in range(4):
                        h = hg * 4 + hh
                        off = (hh // 2) * 512 + (hh % 2) * 196
                        if h % 2 == 0:
                            lhsT = k8[0:64, h // 2, b * N + moff : b * N + moff + mlen]
                            rhs = q8[0:64, h // 2, b * N : b * N + N]
                        else:
                            lhsT = sk8[0:64, h // 2, b * N + moff : b * N + moff + mlen]
                            rhs = sq8[0:64, h // 2, b * N : b * N + N]
                        nc.tensor.matmul(
                            ps[0:mlen, off : off + 196],
                            lhsT=lhsT,
                            rhs=rhs,
                            start=True,
                            stop=True,
                        )
                    src = ps.rearrange("p (k x) -> p k x", k=2)[
                        0:mlen, :, 0:392
                    ].rearrange("p k (h n) -> p k h n", h=2)
                    dst = pt[0:mlen, hg * 4 * N : (hg + 1) * 4 * N].rearrange(
                        "p (k h n) -> p k h n", k=2, h=2
                    )
                    nc.scalar.activation(
                        dst, src, ActFn.Exp, bias=ebias[0:mlen, :], scale=SCALE
                    )

                av_tile = {}

                def av_unit(nc_i, hblk):
                    noff, nlen = NCH[nc_i]
                    if nc_i not in av_tile:
                        av = psB.tile([128, 1024], F32, tag="pB")
                        av_tile[nc_i] = av
                    av = av_tile[nc_i]
                    for h in range(hblk * 3, hblk * 3 + 3):
                        aoff = (h // 8) * 512 + (h % 8) * 64
                        for mc, (moff, mlen) in enumerate(NCH):
                            nc.tensor.matmul(
                                av[0:nlen, aoff : aoff + 64],
                                lhsT=at_sb[
                                    0:mlen, mc, h * N + noff : h * N + noff + nlen
                                ],
                                rhs=vb[0:mlen, mc, h * 65 : h * 65 + 64],
                                start=(mc == 0),
                                stop=(mc == 1),
                            )
                    if hblk == 3:
                        nc.scalar.copy(
                            ob[0:nlen, nc_i, 0:512], av[0:nlen, 0:512]
                        )
                        nc.scalar.copy(
                            ob[0:nlen, nc_i, 512:768], av[0:nlen, 512:768]
                        )

                for mc in range(2):
                    ptile = ptp.tile([128, H * N], BF16, tag="pt")
                    pt_t[(b, mc)] = ptile
                order = [
                    ("st", 0, 0), ("av", 0, 0), ("st", 0, 1), ("av", 0, 1),
                    ("st", 0, 2), ("av", 0, 2), ("st", 1, 0), ("av", 0, 3),
                    ("st", 1, 1), ("av", 1, 0), ("st", 1, 2), ("av", 1, 1),
                    ("av", 1, 2), ("av", 1, 3),
                ]
                for kind, a, bb_ in order:
                    if kind == "st":
                        st_unit(a, bb_)
                    else:
                        av_unit(a, bb_)

            def emit_uo(b, psA):
                vb = vb_t[b]
                ob = ob_t[b]
                for nc_i, (noff, nlen) in enumerate(NCH):
                    rec = small.tile([128, H], F32, tag="rec")
                    tmp = small.tile([128, C], F32, tag="tmp")
                    for half in range(2):
                        uph = psA.tile([128, 512], F32, tag="pA")
                        for h in range(half * 6, half * 6 + 6):
                            uoff = (h % 6) * 65
                            for mc, (moff, mlen) in enumerate(NCH):
                                pt = pt_t[(b, mc)]
                                nc.tensor.matmul(
                                    uph[0:nlen, uoff : uoff + 65],
                                    lhsT=pt[
                                        0:mlen, h * N + noff : h * N + noff + nlen
                                    ],
                                    rhs=vb[0:mlen, mc, h * 65 : h * 65 + 65],
                                    start=(mc == 0),
                                    stop=(mc == 1),
                                )
                        upv = uph[0:nlen, 0:390].rearrange("p (h x) -> p h x", h=6)
                        recv = rec[0:nlen, half * 6 : half * 6 + 6, None]
                        nc.vector.reciprocal(recv, upv[:, :, 64:65])
                        nc.vector.tensor_tensor(
                            tmp[0:nlen, half * 384 : (half + 1) * 384].rearrange(
                                "p (h c) -> p h c", h=6
                            ),
                            upv[:, :, 0:64],
                            recv.to_broadcast((nlen, 6, 64)),
                            AluOp.mult,
                        )
                    nc.vector.tensor_tensor(
                        ob[0:nlen, nc_i, :],
                        tmp[0:nlen, :],
                        ob[0:nlen, nc_i, :],
                        AluOp.add,
                    )

            def emit_tr(b, psA):
                ob = ob_t[b]
                for nc_i, (noff, nlen) in enumerate(NCH):
                    for hp in range(KC):
                        tp = psA.tile([128, 512], BF16, tag="pA")
                        nc.tensor.transpose(
                            tp[:, 0:nlen],
                            in_=ob[0:nlen, nc_i, hp * 128 : (hp + 1) * 128],
                            identity=ident[0:nlen, 0:nlen],
                        )
                        nc.vector.tensor_copy(
                            aot_sb[:, hp, b * N + noff : b * N + noff + nlen],
                            tp[:, 0:nlen],
                        )

            def emit_proj_chunk(mc, pps, tag="pp"):
                moff, mlen = MCS[mc]
                pp = pps.tile([128, 1024], F32, tag=tag)
                for nsl, nw in ((0, 512), (512, 256)):
                    for kc in range(KC):
                        nc.tensor.matmul(
                            pp[0:mlen, nsl : nsl + nw],
                            lhsT=aot_sb[:, kc, moff : moff + mlen],
                            rhs=wp_sb[:, kc, nsl : nsl + nw],
                            start=(kc == 0),
                            stop=(kc == KC - 1),
                        )
                ot = outst.tile([128, C], F32, tag="ot")
                nc.scalar.copy(ot[0:mlen, :], pp[0:mlen, 0:768])
                nc.sync.dma_start(
                    out=out_d[moff : moff + mlen, :], in_=ot[0:mlen, :]
                )

            with (
                tc.tile_pool(name="psA", bufs=2, space=MemorySpace.PSUM) as psA,
                tc.tile_pool(name="psB", bufs=3, space=MemorySpace.PSUM) as psB,
            ):
                # weights/inputs in use-order: qkT needs w8 + xt8 first
                for kp in range(3):
                    nc.sync.dma_start(
                        out=w8_sb[:, kp, :, :].rearrange("p a b -> p (a b)"),
                        in_=w8_d[:, kp * 2 * 2 * C : (kp + 1) * 2 * 2 * C],
                    )
                    nc.sync.dma_start(
                        out=xt8_sb[:, kp, :, :].rearrange("p a b -> p (a b)"),
                        in_=xt8_d[:, kp * 2 * T : (kp + 1) * 2 * T],
                    )
                for kc in range(KC):
                    nc.sync.dma_start(
                        out=xtb_sb[:, kc, :], in_=xtb_d[:, kc * T : (kc + 1) * T]
                    )
                for kc in range(KC):
                    nc.sync.dma_start(
                        out=wv_sb[:, kc, :], in_=wv_d[:, kc * C : (kc + 1) * C]
                    )
                for mc in range(2):
                    for hh in range(2):
                        nc.sync.dma_start(
                            out=at_sb[:, mc, hh * 6 * N : (hh + 1) * 6 * N],
                            in_=at_d[mc, :, hh * 6 * N : (hh + 1) * 6 * N],
                        )

                emit_qkT(psA, psB)
                emit_v(0, psA)
                emit_v(1, psA)
                for b in range(B):
                    if b > 0:
                        emit_uo(b - 1, psA)
                        emit_tr(b - 1, psA)
                    emit_st_av(b, psB)
                    if b + 2 < B:
                        emit_v(b + 2, psA)
                    if b == 1:
                        for kc in range(KC):
                            nc.sync.dma_start(
                                out=wp_sb[:, kc, :],
                                in_=wp_d[:, kc * C : (kc + 1) * C],
                            )
                emit_uo(B - 1, psA)
                emit_tr(B - 1, psA)
                for mc in range(len(MCS)):
                    emit_proj_chunk(mc, psB, tag="pB")

    return nc


_CACHE: dict = {}


def _get_module():
    if "nc" not in _CACHE:
        nc = bacc.Bacc(None, target_bir_lowering=False)
        _emit(nc)
        nc.compile()
        _CACHE["nc"] = nc
    return _CACHE["nc"]


def prepare_core_inputs(x_shard, W_qkv, static_a, W_proj):
    """Build the per-core input map from a [B, N, C] batch shard."""
    bf = ml_dtypes.bfloat16
    f8 = ml_dtypes.float8_e4m3
    xT = np.ascontiguousarray(
        x_shard.reshape(T, C).T
    )  # [768, 1568]
    xt8 = (
        xT.reshape(3, 2, 128, T).transpose(2, 0, 1, 3).reshape(128, 6 * T)
    ).astype(f8)
    xtb = (
        xT.reshape(KC, 128, T).transpose(1, 0, 2).reshape(128, KC * T)
    ).astype(bf)
    return dict(xt8=np.ascontiguousarray(xt8), xtb=np.ascontiguousarray(xtb))


def prepare_shared_inputs(W_qkv, static_a, W_proj):
    bf = ml_dtypes.bfloat16
    f8 = ml_dtypes.float8_e4m3
    w8 = (
        (W_qkv[:, : 2 * C] * W8SCALE)
        .reshape(3, 2, 128, 2 * C)
        .transpose(2, 0, 1, 3)
        .reshape(128, 6 * 2 * C)
    ).astype(f8)
    wv = (
        W_qkv[:, 2 * C :].reshape(KC, 128, C).transpose(1, 0, 2).reshape(128, KC * C)
    ).astype(bf)
    A = static_a[0]  # [H, N, N]
    Am = np.ascontiguousarray(A.transpose(2, 0, 1))  # [m, H, n]
    at_arr = np.zeros((2, 128, H, N), dtype=np.float32)
    at_arr[0] = Am[0:128]
    at_arr[1, 0:68] = Am[128:196]
    at = at_arr.reshape(2, 128, H * N).astype(bf)
    wp = (
        W_proj.reshape(KC, 128, C).transpose(1, 0, 2).reshape(128, KC * C)
    ).astype(bf)
    return dict(
        w8=np.ascontiguousarray(w8),
        wv=np.ascontiguousarray(wv),
        at=np.ascontiguousarray(at),
        wp=np.ascontiguousarray(wp),
    )


_last_results = None


def kernel(x, W_qkv, static_a, W_proj, b_proj):
    global _last_results
    x = np.asarray(x, dtype=np.float32)
    W_qkv = np.asarray(W_qkv, dtype=np.float32)
    static_a = np.asarray(static_a, dtype=np.float32)
    W_proj = np.asarray(W_proj, dtype=np.float32)
    b_proj = np.asarray(b_proj, dtype=np.float32)

    shared = prepare_shared_inputs(W_qkv, static_a, W_proj)
    in_maps = []
    for i in range(N_CORES):
        m = dict(shared)
        m.update(prepare_core_inputs(x[i * B : (i + 1) * B], W_qkv, static_a, W_proj))
        in_maps.append(m)

    nc = _get_module()
    res = run_bass_kernel_spmd(nc, in_maps, core_ids=list(range(N_CORES)))
    _last_results = res
    out = np.concatenate(
        [np.asarray(r["out"]).reshape(B, N, C) for r in res.results], axis=0
    )
    out = out.astype(np.float32)
    if b_proj.any():
        out = out + b_proj.reshape(1, 1, C)
    return out


# revision 6
# speedup vs baseline: 1.0239x; 1.0239x over previous
"""Trainium2 Bass kernel for a 12-head attention block with post-softmax
additive per-head bias.

    qkv = x @ W_qkv                          x: [64, 196, 768]
    attn = softmax(q k^T / 8) + static_a     (bias added AFTER softmax)
    out = (attn @ v) @ W_proj + b_proj

Sharding: data-parallel over batch across 8 NeuronCores (8 batches each).
No collectives. Weights replicated.

Numerics: static_a (~0.9/entry) dominates the softmax probabilities
(~0.005..0.05/entry) in attn@v — the softmax path carries only ~1.2% of
the output norm, so q/k/S can run in fp8-e4m3 while v / A@v / proj stay
bf16. fp8 DoubleRow matmuls stream 2 K-tiles per instruction at 0.5
cyc/row (4x bf16 on the contraction-heavy qkT).

Per-core dataflow:
  prologue: q^T,k^T = W_qk^T @ x^T for ALL 8 local batches at once
            (fp8 DoubleRow, K=768 as 3 pairs of 128) -> fp8 SBUF
  per batch b:
    v(b)   = x_b @ W_v                 (bf16, 65-stride + ones column)
    S^T(b) = k q^T (fp8) -> exp (ACT, scale=1/(8*256), bias=-ln16) -> P~^T
    AV(b)  = A_h @ v  (bf16, per-head static bias term)
    U(b)   = P~ @ [v|1]  (ones column gives softmax row sums)
    O(b)   = U * (1/r) + AV            (DVE)
    O^T(b) via PE transpose -> attn_outT
  out = attn_out @ W_proj  (forward; b_proj added on host — it is zeros)
"""

import math
import os
import sys

_TRN_REPO = "/opt/trn_rl_repo"
if _TRN_REPO not in sys.path:
    sys.path.insert(0, _TRN_REPO)

import numpy as np
import ml_dtypes

import concourse.bass as bass
import concourse.tile as tile
from concourse import bacc, mybir
from concourse.bass import MemorySpace
from concourse.bass_utils import run_bass_kernel_spmd
from concourse.masks import make_identity

BF16 = mybir.dt.bfloat16
F32 = mybir.dt.float32
F8 = mybir.dt.float8e4
DR = mybir.MatmulPerfMode.DoubleRow

N_CORES = 8
BATCH = 64
B = BATCH // N_CORES  # 8 local batches per core
H = 12
D = 64
N = 196
C = 768
T = B * N  # 1568 local tokens
KC = 6  # contraction chunks of 128 over C=768
W8SCALE = 16.0  # host pre-scale on W_q,W_k before fp8 quantization
SCALE = (D ** -0.5) / (W8SCALE * W8SCALE)  # folded into exp
PSHIFT = 16.0  # P~ stored as exp(.)/PSHIFT to stay in fp8 range
EXPBIAS = -math.log(PSHIFT)

TPAD = 208  # aot per-batch column stride (196 + xbar padding)
# per-batch row chunks over N=196
NCH = [(0, 128), (128, 68)]
# qkT free-dim chunks over T (PSUM bank = 512 f32)
QKC = [(i * 512, min(512, T - i * 512)) for i in range((T + 511) // 512)]

AluOp = mybir.AluOpType
ActFn = mybir.ActivationFunctionType


def _emit(nc: bass.Bass):
    # xt8: fp8 x^T in DR layout: xt8[p, kp, kt, t] = x[t, (2kp+kt)*128+p]
    # xtb: bf16 x^T: xtb[p, kc, t] = x[t, kc*128+p]
    # w8:  fp8 16*W_qk: w8[p, kp, kt, f] = 16*W_qkv[(2kp+kt)*128+p, f]
    # wv:  bf16 W_v: wv[p, kc, f] = W_qkv[kc*128+p, 2C+f]
    # at:  bf16 A^T: at[mc, p, h*N+n] = A[h, n, mc*128+p]
    # wp:  bf16 W_proj: wp[p, kc, o] = W_proj[kc*128+p, o]
    xt8_d = nc.declare_dram_parameter("xt8", [128, 6 * T], F8, isOutput=False)
    xtb_d = nc.declare_dram_parameter("xtb", [128, 6 * T], BF16, isOutput=False)
    w8_d = nc.declare_dram_parameter("w8", [128, 6 * 2 * C], F8, isOutput=False)
    wv_d = nc.declare_dram_parameter("wv", [128, KC * C], BF16, isOutput=False)
    at_d = nc.declare_dram_parameter("at", [2, 128, H * N], BF16, isOutput=False)
    wp_d = nc.declare_dram_parameter("wp", [128, KC * C], BF16, isOutput=False)
    # out^T: out_d[o, t] = out[t, o]; host transposes back
    out_d = nc.declare_dram_parameter("out", [C, T], F32, isOutput=True)

    with tile.TileContext(nc) as tc:
        from contextlib import ExitStack

        with ExitStack() as stk:
            const = stk.enter_context(tc.tile_pool(name="const", bufs=1))
            wq = stk.enter_context(tc.tile_pool(name="wq", bufs=1))
            qkp = stk.enter_context(tc.tile_pool(name="qkp", bufs=1))
            vbp = stk.enter_context(tc.tile_pool(name="vbp", bufs=4))
            obp = stk.enter_context(tc.tile_pool(name="obp", bufs=4))
            ptp = stk.enter_context(tc.tile_pool(name="ptp", bufs=6))
            small = stk.enter_context(tc.tile_pool(name="small", bufs=3))
            outst = stk.enter_context(tc.tile_pool(name="outst", bufs=4))
            aotp = stk.enter_context(tc.tile_pool(name="aotp", bufs=1))

            # ---- constants ----
            ebias = const.tile([128, 1], F32)
            nc.vector.memset(ebias, EXPBIAS)

            w8_sb = wq.tile([128, 3, 2, 2 * C], F8)
            wv_sb = wq.tile([128, KC, C], BF16)
            xt8_sb = wq.tile([128, 3, 2, T], F8)
            xtb_sb = wq.tile([128, KC, T], BF16)
            at_sb = const.tile([128, 2, H * N], BF16)
            wp_sb = const.tile([128, KC, C], BF16)
            aot_sb = aotp.tile([128, KC, B * TPAD], BF16)

            # q^T/k^T fp8, feature-major, all batches: q8[p, ft, t]
            q8 = qkp.tile([128, KC, T], F8)
            k8 = qkp.tile([128, KC, T], F8)
            # odd heads' d-rows 64..127 staged to base partition 0
            sq8 = qkp.tile([64, KC, T], F8)
            sk8 = qkp.tile([64, KC, T], F8)

            vb_t = {}
            ob_t = {}
            pt_t = {}

            def emit_qkT(psA, psB):
                # q^T,k^T = W_qk^T @ x^T for all 1568 tokens, fp8 DoubleRow.
                # t-chunk-outer so the first chunk's xt8 DMAs gate only 12
                # groups, not the whole pass.
                for qc, (toff, tlen) in enumerate(QKC):
                    for ft in range(12):
                        dst = q8 if ft < 6 else k8
                        c = ft % 6
                        if (ft + qc) % 2 == 0:
                            pq = psA.tile([128, 512], F32, tag="pA")
                        else:
                            pq = psB.tile([128, 1024], F32, tag="pB")
                        for kp in range(3):
                            nc.tensor.matmul(
                                pq[:, 0:tlen],
                                lhsT=w8_sb[:, kp, :, ft * 128 : ft * 128 + 128],
                                rhs=xt8_sb[:, kp, :, toff : toff + tlen],
                                start=(kp == 0),
                                stop=(kp == 2),
                                perf_mode=DR,
                            )
                        if ft % 2 == 0:
                            nc.vector.tensor_copy(
                                dst[:, c, toff : toff + tlen], pq[:, 0:tlen]
                            )
                        else:
                            nc.scalar.copy(
                                dst[:, c, toff : toff + tlen], pq[:, 0:tlen]
                            )
                        if qc == len(QKC) - 1:
                            # stage odd-head rows down to partition 0 once a
                            # feature-tile is complete
                            stg = sq8 if ft < 6 else sk8
                            nc.sync.dma_start(
                                out=stg[:, c, :], in_=dst[64:128, c, :]
                            )

            def emit_v(b, psA):
                vb = vbp.tile([128, 2, H * 65], BF16, tag="vb")
                vb_t[b] = vb
                for mc, (moff, mlen) in enumerate(NCH):
                    vv = vb[0:mlen, mc, :].rearrange("p (h x) -> p h x", h=H)
                    nc.vector.memset(vv[:, :, 64:65], 1.0)
                    for ns in range(2):
                        ps = psA.tile([128, 512], F32, tag="pA")
                        for kc in range(KC):
                            nc.tensor.matmul(
                                ps[0:mlen, 0:384],
                                lhsT=xtb_sb[
                                    :, kc, b * N + moff : b * N + moff + mlen
                                ],
                                rhs=wv_sb[:, kc, ns * 384 : (ns + 1) * 384],
                                start=(kc == 0),
                                stop=(kc == KC - 1),
                            )
                        nc.vector.tensor_copy(
                            vv[:, ns * 6 : (ns + 1) * 6, 0:64],
                            ps[0:mlen, 0:384].rearrange("p (h c) -> p h c", h=6),
                        )

            def emit_st_av(b, psB):
                # S^T then exp on ACT; A@v blocks interleaved to keep PE fed
                vb = vb_t[b]
                ob = obp.tile([128, 2, C], BF16, tag="ob")
                ob_t[b] = ob
                # zero the xbar pad rows (68..79); base partition must be a
                # multiple of 32, rows 64..67 are rewritten by the av copy
                nc.gpsimd.memset(ob[64:80, 1, :], 0.0)

                def st_unit(mc, hg):
                    moff, mlen = NCH[mc]
                    pt = pt_t[(b, mc)]
                    ps = psB.tile([128, 1024], F32, tag="pB")
                    for hh in range(4):
                        h = hg * 4 + hh
                        off = (hh // 2) * 512 + (hh % 2) * 196
                        if h % 2 == 0:
                            lhsT = k8[0:64, h // 2, b * N + moff : b * N + moff + mlen]
                            rhs = q8[0:64, h // 2, b * N : b * N + N]
                        else:
                            lhsT = sk8[0:64, h // 2, b * N + moff : b * N + moff + mlen]
                            rhs = sq8[0:64, h // 2, b * N : b * N + N]
                        nc.tensor.matmul(
                            ps[0:mlen, off : off + 196],
                            lhsT=lhsT,
                            rhs=rhs,
                            start=True,
                            stop=True,
                        )
                    src = ps.rearrange("p (k x) -> p k x", k=2)[
                        0:mlen, :, 0:392
                    ].rearrange("p k (h n) -> p k h n", h=2)
                    dst = pt[0:mlen, hg * 4 * N : (hg + 1) * 4 * N].rearrange(
                        "p (k h n) -> p k h n", k=2, h=2
                    )
                    nc.scalar.activation(
                        dst, src, ActFn.Exp, bias=ebias[0:mlen, :], scale=SCALE
                    )

                av_tile = {}

                def av_unit(nc_i, hblk):
                    noff, nlen = NCH[nc_i]
                    if nc_i not in av_tile:
                        av = psB.tile([128, 1024], F32, tag="pB")
                        av_tile[nc_i] = av
                    av = av_tile[nc_i]
                    for h in range(hblk * 3, hblk * 3 + 3):
                        aoff = (h // 8) * 512 + (h % 8) * 64
                        for mc, (moff, mlen) in enumerate(NCH):
                            nc.tensor.matmul(
                                av[0:nlen, aoff : aoff + 64],
                                lhsT=at_sb[
                                    0:mlen, mc, h * N + noff : h * N + noff + nlen
                                ],
                                rhs=vb[0:mlen, mc, h * 65 : h * 65 + 64],
                                start=(mc == 0),
                                stop=(mc == 1),
                            )
                    if hblk == 3:
                        nc.scalar.copy(
                            ob[0:nlen, nc_i, 0:512], av[0:nlen, 0:512]
                        )
                        nc.scalar.copy(
                            ob[0:nlen, nc_i, 512:768], av[0:nlen, 512:768]
                        )

                for mc in range(2):
                    ptile = ptp.tile([128, H * N], BF16, tag="pt")
                    pt_t[(b, mc)] = ptile
                order = [
                    ("st", 0, 0), ("av", 0, 0), ("st", 0, 1), ("av", 0, 1),
                    ("st", 0, 2), ("av", 0, 2), ("st", 1, 0), ("av", 0, 3),
                    ("st", 1, 1), ("av", 1, 0), ("st", 1, 2), ("av", 1, 1),
                    ("av", 1, 2), ("av", 1, 3),
                ]
                for kind, a, bb_ in order:
                    if kind == "st":
                        st_unit(a, bb_)
                    else:
                        av_unit(a, bb_)

            def emit_uo(b, psA):
                vb = vb_t[b]
                ob = ob_t[b]
                for nc_i, (noff, nlen) in enumerate(NCH):
                    rec = small.tile([128, H], F32, tag="rec")
                    tmp = small.tile([128, C], F32, tag="tmp")
                    for half in range(2):
                        uph = psA.tile([128, 512], F32, tag="pA")
                        for h in range(half * 6, half * 6 + 6):
                            uoff = (h % 6) * 65
                            for mc, (moff, mlen) in enumerate(NCH):
                                pt = pt_t[(b, mc)]
                                nc.tensor.matmul(
                                    uph[0:nlen, uoff : uoff + 65],
                                    lhsT=pt[
                                        0:mlen, h * N + noff : h * N + noff + nlen
                                    ],
                                    rhs=vb[0:mlen, mc, h * 65 : h * 65 + 65],
                                    start=(mc == 0),
                                    stop=(mc == 1),
                                )
                        upv = uph[0:nlen, 0:390].rearrange("p (h x) -> p h x", h=6)
                        recv = rec[0:nlen, half * 6 : half * 6 + 6, None]
                        nc.vector.reciprocal(recv, upv[:, :, 64:65])
                        nc.vector.tensor_tensor(
                            tmp[0:nlen, half * 384 : (half + 1) * 384].rearrange(
                                "p (h c) -> p h c", h=6
                            ),
                            upv[:, :, 0:64],
                            recv.to_broadcast((nlen, 6, 64)),
                            AluOp.mult,
                        )
                    nc.vector.tensor_tensor(
                        ob[0:nlen, nc_i, :],
                        tmp[0:nlen, :],
                        ob[0:nlen, nc_i, :],
                        AluOp.add,
                    )

            def emit_tr(b):
                # O [n, c] -> O^T [c, n] on the DMA xbar (16x128 tiles).
                # chunk 2 is 68 rows padded to 80; pad rows are zeroed at
                # batch start, pad columns land in aot's per-batch slack.
                ob = ob_t[b]
                nc.sync.dma_start_transpose(
                    aot_sb[:, :, b * TPAD : b * TPAD + 128], ob[0:128, 0, :]
                )
                nc.sync.dma_start_transpose(
                    aot_sb[:, :, b * TPAD + 128 : b * TPAD + 208], ob[0:80, 1, :]
                )

            def emit_proj_bg(bg, pps):
                # out^T[o, t] for batches 2bg..2bg+1: lhsT = W_proj chunk,
                # rhs = aot (skipping the per-batch pad columns)
                for o in range(KC):
                    pp = pps.tile([128, 1024], F32, tag="pB")
                    rhs = aot_sb[:, :, 2 * bg * TPAD : 2 * (bg + 1) * TPAD]
                    rhs = rhs.rearrange("p k (b t) -> p k b t", b=2)[
                        :, :, :, 0:N
                    ]
                    for kc in range(KC):
                        nc.tensor.matmul(
                            pp[:, 0 : 2 * N],
                            lhsT=wp_sb[:, kc, o * 128 : (o + 1) * 128],
                            rhs=rhs[:, kc, :, :],
                            start=(kc == 0),
                            stop=(kc == KC - 1),
                        )
                    ot = outst.tile([128, 2 * N], F32, tag="ot")
                    nc.scalar.copy(ot, pp[:, 0 : 2 * N])
                    nc.sync.dma_start(
                        out=out_d[
                            o * 128 : (o + 1) * 128, 2 * bg * N : 2 * (bg + 1) * N
                        ],
                        in_=ot,
                    )

            with (
                tc.tile_pool(name="psA", bufs=2, space=MemorySpace.PSUM) as psA,
                tc.tile_pool(name="psB", bufs=3, space=MemorySpace.PSUM) as psB,
            ):
                # weights/inputs in use-order: qkT needs w8 + xt8 first
                for kp in range(3):
                    nc.sync.dma_start(
                        out=w8_sb[:, kp, :, :].rearrange("p a b -> p (a b)"),
                        in_=w8_d[:, kp * 2 * 2 * C : (kp + 1) * 2 * 2 * C],
                    )
                for toff, tlen in QKC:
                    for kp in range(3):
                        for kt in range(2):
                            nc.sync.dma_start(
                                out=xt8_sb[:, kp, kt, toff : toff + tlen],
                                in_=xt8_d[
                                    :,
                                    (kp * 2 + kt) * T + toff : (kp * 2 + kt) * T
                                    + toff
                                    + tlen,
                                ],
                            )
                for kc in range(KC):
                    nc.sync.dma_start(
                        out=xtb_sb[:, kc, :], in_=xtb_d[:, kc * T : (kc + 1) * T]
                    )
                for kc in range(KC):
                    nc.sync.dma_start(
                        out=wv_sb[:, kc, :], in_=wv_d[:, kc * C : (kc + 1) * C]
                    )
                for mc in range(2):
                    for hh in range(2):
                        nc.sync.dma_start(
                            out=at_sb[:, mc, hh * 6 * N : (hh + 1) * 6 * N],
                            in_=at_d[mc, :, hh * 6 * N : (hh + 1) * 6 * N],
                        )

                emit_qkT(psA, psB)
                emit_v(0, psA)
                emit_v(1, psA)
                for b in range(B):
                    if b > 0:
                        emit_uo(b - 1, psA)
                        emit_tr(b - 1)
                    emit_st_av(b, psB)
                    if b + 2 < B:
                        emit_v(b + 2, psA)
                    if b == 1:
                        for kc in range(KC):
                            nc.sync.dma_start(
                                out=wp_sb[:, kc, :],
                                in_=wp_d[:, kc * C : (kc + 1) * C],
                            )
                    if b >= 2 and b % 2 == 0:
                        emit_proj_bg(b // 2 - 1, psB)
                emit_uo(B - 1, psA)
                emit_tr(B - 1)
                emit_proj_bg(3, psB)

    return nc


_CACHE: dict = {}


def _get_module():
    if "nc" not in _CACHE:
        nc = bacc.Bacc(None, target_bir_lowering=False)
        _emit(nc)
        nc.compile()
        _CACHE["nc"] = nc
    return _CACHE["nc"]


def prepare_core_inputs(x_shard, W_qkv, static_a, W_proj):
    """Build the per-core input map from a [B, N, C] batch shard."""
    bf = ml_dtypes.bfloat16
    f8 = ml_dtypes.float8_e4m3
    xT = np.ascontiguousarray(
        x_shard.reshape(T, C).T
    )  # [768, 1568]
    xt8 = (
        xT.reshape(3, 2, 128, T).transpose(2, 0, 1, 3).reshape(128, 6 * T)
    ).astype(f8)
    xtb = (
        xT.reshape(KC, 128, T).transpose(1, 0, 2).reshape(128, KC * T)
    ).astype(bf)
    return dict(xt8=np.ascontiguousarray(xt8), xtb=np.ascontiguousarray(xtb))


def prepare_shared_inputs(W_qkv, static_a, W_proj):
    bf = ml_dtypes.bfloat16
    f8 = ml_dtypes.float8_e4m3
    w8 = (
        (W_qkv[:, : 2 * C] * W8SCALE)
        .reshape(3, 2, 128, 2 * C)
        .transpose(2, 0, 1, 3)
        .reshape(128, 6 * 2 * C)
    ).astype(f8)
    wv = (
        W_qkv[:, 2 * C :].reshape(KC, 128, C).transpose(1, 0, 2).reshape(128, KC * C)
    ).astype(bf)
    A = static_a[0]  # [H, N, N]
    Am = np.ascontiguousarray(A.transpose(2, 0, 1))  # [m, H, n]
    at_arr = np.zeros((2, 128, H, N), dtype=np.float32)
    at_arr[0] = Am[0:128]
    at_arr[1, 0:68] = Am[128:196]
    at = at_arr.reshape(2, 128, H * N).astype(bf)
    wp = (
        W_proj.reshape(KC, 128, C).transpose(1, 0, 2).reshape(128, KC * C)
    ).astype(bf)
    return dict(
        w8=np.ascontiguousarray(w8),
        wv=np.ascontiguousarray(wv),
        at=np.ascontiguousarray(at),
        wp=np.ascontiguousarray(wp),
    )


_last_results = None


def kernel(x, W_qkv, static_a, W_proj, b_proj):
    global _last_results
    x = np.asarray(x, dtype=np.float32)
    W_qkv = np.asarray(W_qkv, dtype=np.float32)
    static_a = np.asarray(static_a, dtype=np.float32)
    W_proj = np.asarray(W_proj, dtype=np.float32)
    b_proj = np.asarray(b_proj, dtype=np.float32)

    shared = prepare_shared_inputs(W_qkv, static_a, W_proj)
    in_maps = []
    for i in range(N_CORES):
        m = dict(shared)
        m.update(prepare_core_inputs(x[i * B : (i + 1) * B], W_qkv, static_a, W_proj))
        in_maps.append(m)

    nc = _get_module()
    res = run_bass_kernel_spmd(nc, in_maps, core_ids=list(range(N_CORES)))
    _last_results = res
    out = np.concatenate(
        [np.asarray(r["out"]).reshape(C, B, N).transpose(1, 2, 0) for r in res.results],
        axis=0,
    )
    out = np.ascontiguousarray(out).astype(np.float32)
    if b_proj.any():
        out = out + b_proj.reshape(1, 1, C)
    return out
